# revision 1
# baseline (speedup 1.0000x reference)
"""Trainium2 Bass kernel for AvgReadout-style segment mean + L2 normalize.

reference:
    vsum[i] = sum over edges e with src[e]==i of emb[dst[e]]
    deg[i]  = count of such edges (clamped to >=1)
    out     = l2_normalize(vsum / deg, eps=1e-12)

Key identity: l2_normalize(vsum/deg) == l2_normalize(vsum) whenever deg >= 1
(positive per-row scalar doesn't change direction), and for deg == 0 both are
exactly 0.  So the kernel only needs vsum, never deg.

Distribution: edges are sorted by src on host and sharded by src-range across
8 cores (12500 segments each).  Each core's output slice is disjoint, so no
collectives are needed.

Per core the segments are processed in 98 blocks of 128.  Edge rows are
fetched with dma_gather (int16 indices, so emb is addressed as 4 quarter
tables of 25000 rows); edges are grouped into cells (block, quarter), padded
to whole subtiles of 128 edges.  Cell capacities are maxed across cores so a
single compiled program serves all 8 cores.  Blocks are processed in
superblocks of 4; within a superblock the subtiles are quarter-major so each
dma_gather call (<=1024 indices, the SWDGE ring limit; round-robined over 4
SWDGE queues to parallelize Q7 descriptor generation) reads one quarter
table.  Per subtile a one-hot (edge -> segment-in-block) matrix built on DVE
feeds a PE matmul accumulating into the block's PSUM tile [128 segs, 128
feat].  Pad edges carry an out-of-range srcloc sentinel so their one-hot
column is all zeros.  Epilogue per block: sum-of-squares (ACT Square+accum),
sqrt, clamp 1e-12, reciprocal, scale-copy, DMA out.
"""

import numpy as np
from contextlib import ExitStack

N_SPOT = 100000
D = 128
P = 128
NCORES = 8
SEG_PER_CORE = 12500
NBLK = (SEG_PER_CORE + P - 1) // P  # 98
NQ = 4            # emb quarter tables (int16 index range)
QROWS = N_SPOT // NQ  # 25000
SB = 4            # blocks per superblock (PSUM tiles live concurrently)
CALL_CAP = 8      # subtiles per dma_gather call (1024 idx = SWDGE ring limit)
NQUEUES = 4       # SWDGE queues to round-robin
PAD_SENTINEL = 999.0


def compute_layout(capsub):
    """capsub: [nblk, NQ] int array of per-cell subtile capacities.

    Returns dict with:
      nslots: total subtile slots
      slot_block: per-slot block id
      calls: list of (q, slot0, nsub) dma_gather calls, in slot order
      blk_slots: per-block list of slot ids (ascending)
      sb_list: list of (blocks, calls_idx) per superblock
    """
    capsub = np.asarray(capsub)
    nblk = capsub.shape[0]
    slot_block = []
    calls = []
    blk_slots = [[] for _ in range(nblk)]
    sb_list = []
    slot = 0
    for sb0 in range(0, nblk, SB):
        blocks = list(range(sb0, min(sb0 + SB, nblk)))
        call_lo = len(calls)
        for q in range(NQ):
            run = []  # slots of this (superblock, q) run
            for b in blocks:
                for _ in range(int(capsub[b, q])):
                    slot_block.append(b)
                    blk_slots[b].append(slot)
                    run.append(slot)
                    slot += 1
            for i in range(0, len(run), CALL_CAP):
                chunk = run[i : i + CALL_CAP]
                calls.append((q, chunk[0], len(chunk)))
        sb_list.append((blocks, (call_lo, len(calls))))
    return {
        "nslots": slot,
        "slot_block": slot_block,
        "calls": calls,
        "blk_slots": blk_slots,
        "sb_list": sb_list,
    }


def preprocess(emb, mask, ncores=NCORES, seg_per_core=SEG_PER_CORE, nblk=NBLK,
               nspot=N_SPOT):
    """Sort/shard/pad edges. Returns (in_maps, capsub, layout)."""
    qrows = nspot // NQ
    emb = np.ascontiguousarray(np.asarray(emb, dtype=np.float32))
    emb16 = emb.astype(np.float16)
    mask = np.asarray(mask)
    src = mask[0].astype(np.int64, copy=False)
    dst = mask[1].astype(np.int64, copy=False)

    order = np.argsort(src, kind="stable")
    src_s = src[order].astype(np.int32)
    dst_s = dst[order].astype(np.int32)

    core_bounds = np.searchsorted(
        src_s, (seg_per_core * np.arange(ncores + 1)).astype(np.int32)
    )

    percore = []
    cnts = np.zeros((ncores, nblk * NQ), np.int64)
    for k in range(ncores):
        lo, hi = int(core_bounds[k]), int(core_bounds[k + 1])
        s = src_s[lo:hi] - seg_per_core * k
        d = dst_s[lo:hi]
        cell = (s >> 7) * NQ + d // qrows
        o = np.lexsort((d, cell))
        s, d, cell = s[o], d[o], cell[o]
        cnts[k] = np.bincount(cell, minlength=nblk * NQ)
        percore.append((s, d, cell))

    capsub = (-(-cnts.max(axis=0) // P)).reshape(nblk, NQ).astype(np.int64)
    layout = compute_layout(capsub)
    nslots = layout["nslots"]

    # slot base per cell, following the layout's slot order
    cell_slot0 = np.zeros(nblk * NQ, np.int64)
    nxt = np.zeros(nblk * NQ, np.int64)
    slot_of_cell = {}
    # reconstruct per-cell slot bases: slots are assigned per (sb, q, b) in
    # capsub order; walk the same order.
    slot = 0
    for sb0 in range(0, nblk, SB):
        for q in range(NQ):
            for b in range(sb0, min(sb0 + SB, nblk)):
                cell_slot0[b * NQ + q] = slot
                slot += int(capsub[b, q])
    assert slot == nslots

    iota = np.broadcast_to(np.arange(P, dtype=np.float16)[None, :], (P, P)).copy()

    in_maps = []
    for k in range(ncores):
        s, d, cell = percore[k]
        cum = np.zeros(nblk * NQ, np.int64)
        cc = cnts[k]
        cum[1:] = np.cumsum(cc)[:-1]
        rank = np.arange(len(s), dtype=np.int64) - cum[cell]
        pos = cell_slot0[cell] * P + rank  # global edge position

        srcloc = np.full(nslots * P, PAD_SENTINEL, np.float16)
        srcloc[pos] = (s & 127).astype(np.float16)
        dloc = np.zeros(nslots * P, np.int16)
        dloc[pos] = (d % qrows).astype(np.int16)

        # srcloc tile [p, slot] = value of edge (slot, p)
        srcloc_t = np.ascontiguousarray(srcloc.reshape(nslots, P).T)
        # idx16 [j%16, slot*8 + j//16] = dloc of edge (slot, j), replicated
        # across the 8 partition groups for the Q7 ucode.
        idx_blk = np.ascontiguousarray(
            dloc.reshape(nslots * 8, 16).T
        )  # [16, nslots*8]
        idx16 = np.tile(idx_blk, (8, 1))
        in_maps.append(
            {"emb": emb16, "srcloc": srcloc_t, "dstidx": idx16, "iota": iota}
        )
    return in_maps, capsub, layout


def build_program(capsub, layout, nblk=NBLK, nspot=N_SPOT, d=D, repeats=1):
    import concourse.bass as bass
    import concourse.tile as tile
    from concourse import bacc, mybir

    qrows = nspot // NQ
    nslots = layout["nslots"]
    calls = layout["calls"]
    blk_slots = layout["blk_slots"]
    sb_list = layout["sb_list"]

    nc = bacc.Bacc(
        "TRN2", target_bir_lowering=False, debug=False, num_swdge_queues=NQUEUES
    )
    emb_t = nc.dram_tensor("emb", [nspot, d], mybir.dt.float16, kind="ExternalInput")
    srcloc_t = nc.dram_tensor(
        "srcloc", [P, nslots], mybir.dt.float16, kind="ExternalInput"
    )
    dstidx_t = nc.dram_tensor(
        "dstidx", [P, nslots * 8], mybir.dt.int16, kind="ExternalInput"
    )
    iota_t = nc.dram_tensor("iota", [P, P], mybir.dt.float16, kind="ExternalInput")
    out_t = nc.dram_tensor(
        "out", [nblk * P, d], mybir.dt.float32, kind="ExternalOutput"
    )

    # slot -> (call idx, position within call)
    slot_call = [None] * nslots
    for ci, (q, s0, nsub) in enumerate(calls):
        for t in range(nsub):
            slot_call[s0 + t] = (ci, t)

    with tile.TileContext(nc) as tc, ExitStack() as ctx:
        consts = ctx.enter_context(tc.tile_pool(name="consts", bufs=1))
        gpool = ctx.enter_context(tc.tile_pool(name="gather", bufs=24))
        ohpool = ctx.enter_context(tc.tile_pool(name="onehot", bufs=24))
        spool = ctx.enter_context(tc.tile_pool(name="scratch", bufs=4))
        opool = ctx.enter_context(tc.tile_pool(name="outs", bufs=4))
        ppool = ctx.enter_context(tc.tile_pool(name="psum", bufs=8, space="PSUM"))

        srcloc_sb = consts.tile([P, nslots], mybir.dt.float16)
        nc.sync.dma_start(srcloc_sb[:], srcloc_t.ap())
        dstidx_sb = consts.tile([P, nslots * 8], mybir.dt.int16)
        nc.sync.dma_start(dstidx_sb[:], dstidx_t.ap())
        iota_sb = consts.tile([P, P], mybir.dt.float16)
        nc.sync.dma_start(iota_sb[:], iota_t.ap())

        out_ap = out_t.ap()
        emb_ap = emb_t.ap()
        callno = 0
        for rep in range(repeats):
            for blocks, (clo, chi) in sb_list:
                gtiles = {}
                ohtiles = {}
                for ci in range(clo, chi):
                    q, s0, nsub = calls[ci]
                    gt = gpool.tile([P, CALL_CAP * d], mybir.dt.float16, tag="gt")
                    nc.gpsimd.dma_gather(
                        out_ap=gt[:, : nsub * d].rearrange(
                            "p (c e) -> p c e", e=d
                        ),
                        in_ap=emb_ap[q * qrows : (q + 1) * qrows, :],
                        idxs_ap=dstidx_sb[:, s0 * 8 : (s0 + nsub) * 8],
                        num_idxs=nsub * P,
                        num_idxs_reg=nsub * P,
                        elem_size=d,
                        single_packet=False,
                        queue_num=callno % NQUEUES,
                    )
                    gtiles[ci] = gt
                    callno += 1
                    # one batched one-hot build for the call's subtiles:
                    # oh[p, t, c] = (iota[p, c] == srcloc[p, s0+t])
                    oh = ohpool.tile([P, CALL_CAP * P], mybir.dt.float16, tag="oh")
                    oh3 = oh[:, : nsub * P].rearrange("p (t c) -> p t c", c=P)
                    iota_b = bass.AP(
                        iota_sb[:].tensor,
                        iota_sb[:].offset,
                        [iota_sb[:].ap[0], [0, nsub], [1, P]],
                    )
                    srl = srcloc_sb[:, s0 : s0 + nsub]
                    srl_b = bass.AP(
                        srl.tensor, srl.offset, [srl.ap[0], [1, nsub], [0, P]]
                    )
                    nc.vector.tensor_tensor(
                        out=oh3,
                        in0=iota_b,
                        in1=srl_b,
                        op=mybir.AluOpType.is_equal,
                    )
                    ohtiles[ci] = oh
                for b in blocks:
                    slots = blk_slots[b]
                    if not slots:
                        ot = opool.tile([P, d], mybir.dt.float32)
                        nc.vector.memset(ot[:], 0.0)
                        nc.sync.dma_start(out_ap[b * P : (b + 1) * P, :], ot[:])
                        continue
                    ps = ppool.tile([P, d], mybir.dt.float32, space="PSUM")
                    for i, sl in enumerate(slots):
                        ci, t = slot_call[sl]
                        nc.tensor.matmul(
                            ps[:],
                            lhsT=ohtiles[ci][:, t * P : (t + 1) * P],
                            rhs=gtiles[ci][:, t * d : (t + 1) * d],
                            start=(i == 0),
                            stop=(i == len(slots) - 1),
                        )
                    sq = spool.tile([P, d], mybir.dt.float32)
                    ss = spool.tile([P, 1], mybir.dt.float32)
                    nc.scalar.activation(
                        sq[:],
                        ps[:],
                        mybir.ActivationFunctionType.Square,
                        accum_out=ss[:],
                    )
                    nrm = spool.tile([P, 1], mybir.dt.float32)
                    nc.scalar.activation(
                        nrm[:], ss[:], mybir.ActivationFunctionType.Sqrt
                    )
                    nc.vector.tensor_scalar(
                        out=nrm[:],
                        in0=nrm[:],
                        scalar1=1e-12,
                        scalar2=None,
                        op0=mybir.AluOpType.max,
                    )
                    nc.vector.reciprocal(nrm[:], nrm[:])
                    ot = opool.tile([P, d], mybir.dt.float32)
                    nc.scalar.activation(
                        ot[:],
                        ps[:],
                        mybir.ActivationFunctionType.Copy,
                        scale=nrm[:],
                    )
                    nc.sync.dma_start(out_ap[b * P : (b + 1) * P, :], ot[:])

    nc.compile()
    return nc


_PROGRAM_CACHE = {}


def _get_program(capsub, layout):
    key = capsub.tobytes()
    if key not in _PROGRAM_CACHE:
        _PROGRAM_CACHE[key] = build_program(capsub, layout)
    return _PROGRAM_CACHE[key]


def kernel(**inputs):
    emb = inputs["emb"]
    mask = inputs["mask"]
    in_maps, capsub, layout = preprocess(emb, mask)
    nc = _get_program(capsub, layout)

    from concourse.bass_utils import run_bass_kernel_spmd

    res = run_bass_kernel_spmd(nc, in_maps, core_ids=list(range(NCORES)))
    out = np.empty((N_SPOT, D), np.float32)
    for k in range(NCORES):
        out[k * SEG_PER_CORE : (k + 1) * SEG_PER_CORE] = res.results[k]["out"][
            :SEG_PER_CORE
        ]
    return out



# revision 5
# speedup vs baseline: 282.3273x; 282.3273x over previous
"""Trainium2 Bass kernel for AvgReadout-style segment mean + L2 normalize.

reference:
    vsum[i] = sum over edges e with src[e]==i of emb[dst[e]]
    deg[i]  = count of such edges (clamped to >=1)
    out     = l2_normalize(vsum / deg, eps=1e-12)

Key identity: l2_normalize(vsum/deg) == l2_normalize(vsum) whenever deg >= 1
(positive per-row scalar doesn't change direction), and for deg == 0 both are
exactly 0.  So the kernel only needs vsum, never deg.

Distribution: edges are sorted by src on host and sharded by src-range across
8 cores (12500 segments each).  Each core's output slice is disjoint, so no
collectives are needed.

Per core the 12500 segments form 98 blocks of 128, grouped into superblocks
of SB=4 blocks.  Edge rows are fetched with dma_gather (int16 indices, so emb
is addressed as 4 quarter tables of 25000 rows).  Edges are packed per
(superblock, quarter) run: block-major, dst-ascending, rounded up to whole
128-edge subtiles only at run granularity, so a boundary subtile may hold
edges of two adjacent blocks.  Each block j of the superblock builds its
one-hot against iota window [128j, 128j+128) over srcloc values relative to
the superblock (s - 512*sbi plus a 999 pad sentinel), so foreign edges and
pads contribute zero columns.  Pad gathers point at spread-out rows (an
all-same-row pad pattern serializes on one HBM bank, measured ~4x slower).

Runs are chunked into dma_gather calls of CALL_CAP subtiles round-robined
over 4 SWDGE queues, with a descriptor ring deep enough to keep several
calls in flight per queue -- the gather is latency-bound per call chain
(1 queue measured 3x slower), so pipeline depth is what matters.

Per block a PE matmul chain accumulates one-hot^T @ gathered into a PSUM
tile [128 segs, 128 feat].  Epilogue per block: sum-of-squares (ACT
Square+accum), sqrt, clamp 1e-12, reciprocal, scale-copy, DMA out.
"""

import numpy as np
from contextlib import ExitStack

N_SPOT = 100000
D = 128
P = 128
NCORES = 8
SEG_PER_CORE = 12500
NBLK = (SEG_PER_CORE + P - 1) // P  # 98
NQ = 4                # emb quarter tables (int16 index range)
QROWS = N_SPOT // NQ  # 25000
SB = 4                # blocks per superblock (PSUM tiles live concurrently)
NSB = (NBLK + SB - 1) // SB  # 25
NQUEUES = 4
CALL_CAP = 9          # subtiles per dma_gather call
SCRATCH = 49152       # SWDGE descriptor ring: 3072 descs (~2.7 calls deep)
PAD_SENTINEL = 999.0


def preprocess(emb, mask, spread_pads=True):
    """Sort/shard/pad edges into the merged-run layout.

    Returns (in_maps, meta); meta carries the per-(superblock, quarter) run
    capacities and per-block subtile spans (unioned across cores so one
    compiled program serves all 8 cores).
    """
    emb = np.ascontiguousarray(np.asarray(emb, dtype=np.float32))
    emb16 = emb.astype(np.float16)
    mask = np.asarray(mask)
    src = mask[0].astype(np.int64, copy=False)
    dst = mask[1].astype(np.int64, copy=False)

    order = np.argsort(src, kind="stable")
    src_s = src[order].astype(np.int32)
    dst_s = dst[order].astype(np.int32)
    core_bounds = np.searchsorted(
        src_s, (SEG_PER_CORE * np.arange(NCORES + 1)).astype(np.int32)
    )

    percore = []
    cnt3 = np.zeros((NCORES, NSB, NQ, SB), np.int64)
    for k in range(NCORES):
        lo, hi = int(core_bounds[k]), int(core_bounds[k + 1])
        s = src_s[lo:hi] - SEG_PER_CORE * k
        d = dst_s[lo:hi]
        b = s >> 7
        key = ((b // SB) * NQ + d // QROWS) * SB + b % SB
        o = np.lexsort((d, key))
        s, d, key = s[o], d[o], key[o]
        cnt3[k] = np.bincount(key, minlength=NSB * NQ * SB).reshape(NSB, NQ, SB)
        percore.append((s, d, key))

    cnt2 = cnt3.sum(axis=3)
    cap2 = -(-cnt2.max(axis=0) // P)             # [NSB, NQ] subtiles per run
    off3 = np.cumsum(cnt3, axis=3) - cnt3
    end3 = off3 + cnt3
    span_lo = np.where(cnt3 > 0, off3 // P, np.iinfo(np.int64).max).min(axis=0)
    span_hi = np.where(cnt3 > 0, -(-end3 // P), 0).max(axis=0)

    run_slot0 = np.zeros((NSB, NQ), np.int64)
    slot = 0
    for sbi in range(NSB):
        for q in range(NQ):
            run_slot0[sbi, q] = slot
            slot += int(cap2[sbi, q])
    nslots = slot

    in_maps = []
    for k in range(NCORES):
        s, d, key = percore[k]
        cnt2k = cnt3[k].sum(axis=2)
        runkey = key // SB
        run_starts = np.zeros(NSB * NQ, np.int64)
        run_starts[1:] = np.cumsum(cnt2k.reshape(-1))[:-1]
        rank = np.arange(len(s), dtype=np.int64) - run_starts[runkey]
        pos = run_slot0.reshape(-1)[runkey] * P + rank

        srcloc = np.full(nslots * P, PAD_SENTINEL, np.float16)
        srcloc[pos] = (s - 512 * (runkey // NQ)).astype(np.float16)
        dloc = np.zeros(nslots * P, np.int16)
        if spread_pads:
            filled = np.zeros(nslots * P, bool)
            filled[pos] = True
            padpos = np.flatnonzero(~filled)
            dloc[padpos] = ((padpos * 2654435761) % QROWS).astype(np.int16)
        dloc[pos] = (d % QROWS).astype(np.int16)

        # srcloc tile [p, slot]; idx16 wraps each subtile's 128 indices into
        # 16 partitions and replicates across the 8 partition groups for Q7
        srcloc_t = np.ascontiguousarray(srcloc.reshape(nslots, P).T)
        idx_blk = np.ascontiguousarray(dloc.reshape(nslots * 8, 16).T)
        idx16 = np.tile(idx_blk, (8, 1))
        iota4 = np.broadcast_to(
            np.arange(SB * P, dtype=np.float16)[None, :], (P, SB * P)
        ).copy()
        in_maps.append(
            {"emb": emb16, "srcloc": srcloc_t, "dstidx": idx16, "iota": iota4}
        )

    meta = {
        "cap2": cap2,
        "span_lo": span_lo,
        "span_hi": span_hi,
        "run_slot0": run_slot0,
        "nslots": nslots,
    }
    return in_maps, meta


def build_program(meta, repeats=1, call_cap=CALL_CAP, nqueues=NQUEUES,
                  scratch=SCRATCH, batch_epilogue=False):
    import concourse.bass as bass
    import concourse.tile as tile
    from concourse import bacc, mybir

    cap2 = np.asarray(meta["cap2"])
    span_lo = np.asarray(meta["span_lo"])
    span_hi = np.asarray(meta["span_hi"])
    run_slot0 = np.asarray(meta["run_slot0"])
    nslots = int(meta["nslots"])
    gtw = call_cap
    ohw = int((span_hi - span_lo).clip(min=0).max())

    nc = bacc.Bacc(
        "TRN2", target_bir_lowering=False, debug=False,
        num_swdge_queues=nqueues, dynamic_dma_scratch_size=scratch,
    )
    if batch_epilogue:
        # scalar bias for ACT Sqrt needs a registered [128,1] const AP
        _eps = nc.alloc_sbuf_tensor("const-eps", [P, 1], mybir.dt.float32)
        nc.gpsimd.memset(_eps.ap(), 1e-24)
        nc.const_aps.aps[(mybir.dt.float32, 1e-24)] = _eps.ap()
        nc.all_engine_barrier()
    emb_t = nc.dram_tensor("emb", [N_SPOT, D], mybir.dt.float16,
                           kind="ExternalInput")
    srcloc_t = nc.dram_tensor("srcloc", [P, nslots], mybir.dt.float16,
                              kind="ExternalInput")
    dstidx_t = nc.dram_tensor("dstidx", [P, nslots * 8], mybir.dt.int16,
                              kind="ExternalInput")
    iota_t = nc.dram_tensor("iota", [P, SB * P], mybir.dt.float16,
                            kind="ExternalInput")
    out_t = nc.dram_tensor("out", [NBLK * P, D], mybir.dt.float32,
                           kind="ExternalOutput")

    with tile.TileContext(nc) as tc, ExitStack() as ctx:
        consts = ctx.enter_context(tc.tile_pool(name="consts", bufs=1))
        gpool = ctx.enter_context(tc.tile_pool(name="gather", bufs=8))
        ohpool = ctx.enter_context(tc.tile_pool(name="onehot", bufs=24))
        spool = ctx.enter_context(tc.tile_pool(name="scratch", bufs=4))
        opool = ctx.enter_context(tc.tile_pool(name="outs", bufs=4))
        ppool = ctx.enter_context(tc.tile_pool(name="psum", bufs=8,
                                               space="PSUM"))

        srcloc_sb = consts.tile([P, nslots], mybir.dt.float16)
        nc.sync.dma_start(srcloc_sb[:], srcloc_t.ap())
        dstidx_sb = consts.tile([P, nslots * 8], mybir.dt.int16)
        nc.sync.dma_start(dstidx_sb[:], dstidx_t.ap())
        iota_sb = consts.tile([P, SB * P], mybir.dt.float16)
        nc.sync.dma_start(iota_sb[:], iota_t.ap())

        out_ap = out_t.ap()
        emb_ap = emb_t.ap()
        callno = 0
        for rep in range(repeats):
            for sbi in range(NSB):
                nb = min(SB, NBLK - sbi * SB)
                gts = {}
                ohs = {}
                for q in range(NQ):
                    cap = int(cap2[sbi, q])
                    if cap == 0:
                        continue
                    s0 = int(run_slot0[sbi, q])
                    for c0 in range(0, cap, call_cap):
                        nsub = min(call_cap, cap - c0)
                        gt = gpool.tile([P, gtw * D], mybir.dt.float16,
                                        tag="gt")
                        nc.gpsimd.dma_gather(
                            out_ap=gt[:, : nsub * D].rearrange(
                                "p (c e) -> p c e", e=D
                            ),
                            in_ap=emb_ap[q * QROWS : (q + 1) * QROWS, :],
                            idxs_ap=dstidx_sb[
                                :, (s0 + c0) * 8 : (s0 + c0 + nsub) * 8
                            ],
                            num_idxs=nsub * P,
                            num_idxs_reg=nsub * P,
                            elem_size=D,
                            single_packet=False,
                            queue_num=callno % nqueues,
                        )
                        gts[(q, c0 // call_cap)] = gt
                        callno += 1
                    for j in range(nb):
                        lo = int(span_lo[sbi, q, j])
                        hi = int(span_hi[sbi, q, j])
                        if lo >= hi:
                            continue
                        w = hi - lo
                        oh = ohpool.tile([P, ohw * P], mybir.dt.float16,
                                         tag="oh")
                        oh3 = oh[:, : w * P].rearrange("p (t c) -> p t c", c=P)
                        iot = iota_sb[:, j * P : (j + 1) * P]
                        iota_b = bass.AP(
                            iot.tensor, iot.offset,
                            [iot.ap[0], [0, w], [1, P]],
                        )
                        srl = srcloc_sb[:, s0 + lo : s0 + hi]
                        srl_b = bass.AP(
                            srl.tensor, srl.offset,
                            [srl.ap[0], [1, w], [0, P]],
                        )
                        nc.vector.tensor_tensor(
                            out=oh3, in0=iota_b, in1=srl_b,
                            op=mybir.AluOpType.is_equal,
                        )
                        ohs[(q, j)] = oh

                def block_chain(j):
                    chain = []
                    for q in range(NQ):
                        if int(cap2[sbi, q]) == 0:
                            continue
                        lo = int(span_lo[sbi, q, j])
                        hi = int(span_hi[sbi, q, j])
                        for t in range(lo, hi):
                            chain.append((q, t, lo))
                    return chain

                def run_chain(ps, j, chain):
                    for i, (q, t, lo) in enumerate(chain):
                        gt = gts[(q, t // call_cap)]
                        tt = t % call_cap
                        nc.tensor.matmul(
                            ps[:],
                            lhsT=ohs[(q, j)][
                                :, (t - lo) * P : (t - lo + 1) * P
                            ],
                            rhs=gt[:, tt * D : (tt + 1) * D],
                            start=(i == 0),
                            stop=(i == len(chain) - 1),
                        )

                if batch_epilogue:
                    ss4 = spool.tile([P, SB], mybir.dt.float32, tag="ss4")
                    ot4 = opool.tile([P, SB * D], mybir.dt.float32, tag="ot4")
                    pss = {}
                    for j in range(nb):
                        chain = block_chain(j)
                        if not chain:
                            nc.vector.memset(ot4[:, j * D : (j + 1) * D], 0.0)
                            nc.vector.memset(ss4[:, j : j + 1], 1.0)
                            continue
                        ps = ppool.tile([P, D], mybir.dt.float32, space="PSUM")
                        run_chain(ps, j, chain)
                        pss[j] = ps
                        sq = spool.tile([P, D], mybir.dt.float32, tag="sq")
                        nc.scalar.activation(
                            sq[:], ps[:],
                            mybir.ActivationFunctionType.Square,
                            accum_out=ss4[:, j : j + 1],
                        )
                    nrm4 = spool.tile([P, SB], mybir.dt.float32, tag="nrm4")
                    nc.scalar.activation(
                        nrm4[:, :nb], ss4[:, :nb],
                        mybir.ActivationFunctionType.Sqrt,
                        bias=1e-24,
                    )
                    nc.vector.reciprocal(nrm4[:, :nb], nrm4[:, :nb])
                    for j, ps in pss.items():
                        nc.scalar.activation(
                            ot4[:, j * D : (j + 1) * D], ps[:],
                            mybir.ActivationFunctionType.Copy,
                            scale=nrm4[:, j : j + 1],
                        )
                    dst = out_ap[sbi * SB * P : (sbi * SB + nb) * P, :]
                    nc.sync.dma_start(
                        dst.rearrange("(j p) f -> p j f", p=P),
                        ot4[:, : nb * D].rearrange("p (j f) -> p j f", f=D),
                    )
                    continue

                for j in range(nb):
                    b = sbi * SB + j
                    chain = block_chain(j)
                    if not chain:
                        ot = opool.tile([P, D], mybir.dt.float32)
                        nc.vector.memset(ot[:], 0.0)
                        nc.sync.dma_start(out_ap[b * P : (b + 1) * P, :],
                                          ot[:])
                        continue
                    ps = ppool.tile([P, D], mybir.dt.float32, space="PSUM")
                    run_chain(ps, j, chain)
                    sq = spool.tile([P, D], mybir.dt.float32)
                    ss = spool.tile([P, 1], mybir.dt.float32)
                    nc.scalar.activation(
                        sq[:], ps[:],
                        mybir.ActivationFunctionType.Square,
                        accum_out=ss[:],
                    )
                    nrm = spool.tile([P, 1], mybir.dt.float32)
                    nc.scalar.activation(
                        nrm[:], ss[:], mybir.ActivationFunctionType.Sqrt
                    )
                    nc.vector.tensor_scalar(
                        out=nrm[:], in0=nrm[:], scalar1=1e-12, scalar2=None,
                        op0=mybir.AluOpType.max,
                    )
                    nc.vector.reciprocal(nrm[:], nrm[:])
                    ot = opool.tile([P, D], mybir.dt.float32)
                    nc.scalar.activation(
                        ot[:], ps[:],
                        mybir.ActivationFunctionType.Copy,
                        scale=nrm[:],
                    )
                    nc.sync.dma_start(out_ap[b * P : (b + 1) * P, :], ot[:])

    nc.compile()
    return nc


_PROGRAM_CACHE = {}


def _meta_key(meta):
    return (meta["cap2"].tobytes(), meta["span_lo"].tobytes(),
            meta["span_hi"].tobytes())


def _get_program(meta):
    key = _meta_key(meta)
    if key not in _PROGRAM_CACHE:
        _PROGRAM_CACHE[key] = build_program(meta)
    return _PROGRAM_CACHE[key]


def kernel(**inputs):
    emb = inputs["emb"]
    mask = inputs["mask"]
    in_maps, meta = preprocess(emb, mask)
    nc = _get_program(meta)

    from concourse.bass_utils import run_bass_kernel_spmd

    res = run_bass_kernel_spmd(nc, in_maps, core_ids=list(range(NCORES)))
    out = np.empty((N_SPOT, D), np.float32)
    for k in range(NCORES):
        out[k * SEG_PER_CORE : (k + 1) * SEG_PER_CORE] = res.results[k]["out"][
            :SEG_PER_CORE
        ]
    return out
